# revision 8
# baseline (speedup 1.0000x reference)
"""Trainium2 Bass kernel for the MultiLatentAttention (dense transformer) block.

Computes, for x:(4,2048,2048), mask:(4,1,2048,2048):
    q/k/v = x @ W{q,k,v} + b  (per-head, head_dim=128, 16 heads)
    q,k <- interleaved RoPE
    attn = softmax(q k^T / sqrt(2048)) * mask
    out  = (attn @ v) @ Wo + bo

Sharding: 8 cores = 4 batches x 2 head-groups (8 heads each). Each core
computes its batch's q/k/v for its 8 heads, attention, and a partial
o-projection (row-parallel over Wo). Host sums the two partials per batch
and adds bo. No device collectives.

Numerics / layout:
 - q/k projections run in fp8e4 with perf_mode=DoubleRow (256-deep
   contraction per pass, ~1.8x the fp16 matmul rate). Wq/Wk are scaled
   x32 on host so their values sit in e4m3's normal range; the x1024
   scores scale is folded into the exp() scale. Simulated end-to-end
   max-rel-err of this scheme is 1.3e-2 (gate: 2e-2); everything else
   runs in fp16 which alone sims at 5.4e-4.
 - v projection / scores / attn@v / o-projection all use fp16 operands
   (fp32 PSUM accumulate). fp16 halves LDWEIGHTS time vs fp32r and all
   SBUF/DMA traffic.
 - RoPE interleaved pairs are de-interleaved by permuting W{q,k} columns
   per head (even dims -> partitions 0..63, odd -> 64..127); q.k is
   invariant under a shared head-dim permutation and v/Wo are left
   unpermuted. Rotation sign is folded into the sin table.
 - softmax has no max-subtraction (scores are O(1)); the denominator is
   accumulated across k-blocks on DVE in fp16, then one pair of
   accumulating ones-stationary matmuls per (qc, head) produces the
   partition-broadcast sum (vs baseline's 16 ones-matmuls).
 - Everything stays resident in SBUF: q/k (fp16, 8MB), v (fp16, 4MB),
   per-qc mask tiles and attention outputs. Only the fp16 output
   partials are spilled (SWDGE so stores never block the load queue).
 - Attention loops qc (512 query cols) outer, heads inner; the o-proj
   for those 4 row-blocks runs right after each qc, giving the scalar
   engine (exp) slack to catch up while the PE does matmuls with no
   activation dependency.
"""

import numpy as np

B, S, H, NH = 4, 2048, 2048, 16
D = 128            # head dim
G = 2              # head groups (tensor-parallel)
HL = NH // G       # heads per core = 8
P = 128
KO = H // P        # 16 contraction blocks
KOP = KO // 2      # 8 DoubleRow contraction pairs
SB = S // P        # 16 sequence blocks
NQ = S // 512      # 4 query-column chunks
ROPE_BASE = 10000.0
WSCALE = 32.0      # host scale on Wq/Wk (and bq/bk) for fp8 range
SCALE = 1.0 / np.sqrt(np.float32(H))
EXP_SCALE = float(SCALE / (WSCALE * WSCALE))

_CACHE = {}


def _build_program(with_bv):
    import concourse.mybir as mybir
    import concourse.tile as tile
    from concourse import bacc

    f32 = mybir.dt.float32
    f16 = mybir.dt.float16
    f8 = mybir.dt.float8e4
    AF = mybir.ActivationFunctionType
    DR = mybir.MatmulPerfMode.DoubleRow
    ADD = mybir.AluOpType.add
    MULT = mybir.AluOpType.mult

    nc = bacc.Bacc("TRN2", num_devices=8, debug=False, num_swdge_queues=4)

    xtA = nc.dram_tensor("xtA", [P, KOP * 2 * S], f8, kind="ExternalInput")
    xtB = nc.dram_tensor("xtB", [P, KO * S], f16, kind="ExternalInput")
    wq8 = nc.dram_tensor("wq8", [HL, P, KOP * 2 * D], f8, kind="ExternalInput")
    wk8 = nc.dram_tensor("wk8", [HL, P, KOP * 2 * D], f8, kind="ExternalInput")
    bqs = nc.dram_tensor("bqs", [P, HL], f32, kind="ExternalInput")
    bks = nc.dram_tensor("bks", [P, HL], f32, kind="ExternalInput")
    wv = nc.dram_tensor("wv", [H, HL * D], f16, kind="ExternalInput")
    bv = nc.dram_tensor("bv", [P, HL * D], f16, kind="ExternalInput")
    wo = nc.dram_tensor("wo", [P, HL * H], f16, kind="ExternalInput")
    cosP = nc.dram_tensor("cosP", [P, S], f16, kind="ExternalInput")
    sinP = nc.dram_tensor("sinP", [P, S], f16, kind="ExternalInput")
    maskT = nc.dram_tensor("maskT", [S, S], f16, kind="ExternalInput")
    ones_d = nc.dram_tensor("ones", [P, P], f16, kind="ExternalInput")

    out = nc.dram_tensor("out", [S, H], f16, kind="ExternalOutput")

    xtA_r = xtA.rearrange("p (kp q i c) -> p kp q i c", kp=KOP, q=NQ, i=2)
    xtB_r = xtB.rearrange("p (ko s) -> p ko s", s=S)
    wv_r = wv.rearrange("(ko p) n -> ko p n", p=P)
    wo_r = wo.rearrange("p (h n) -> p h n", n=H)
    maskT_r = maskT.rearrange("(ko p) s -> ko p s", p=P)
    out_r = out.rearrange("(mo p) n -> mo p n", p=P)

    with tile.TileContext(nc) as tc:
        with (
            tc.tile_pool(name="vres", bufs=1) as vres_pool,
            tc.tile_pool(name="qkres", bufs=1) as qkres_pool,
            tc.tile_pool(name="cs", bufs=1) as cs_pool,
        ):
            v_sb = vres_pool.tile([P, SB, HL * D], f16, name="v_sb")
            qt_t = [
                qkres_pool.tile([P, S], f16, name=f"qt{h}", tag=f"qt{h}")
                for h in range(HL)
            ]
            kt_t = [
                qkres_pool.tile([P, S], f16, name=f"kt{h}", tag=f"kt{h}")
                for h in range(HL)
            ]
            ones_sb = cs_pool.tile([P, P], f16, name="ones_sb")

            # ---------------- phase B: q/k projections (fp8 DoubleRow) ----
            with (
                tc.tile_pool(name="xta", bufs=1) as xta_pool,
                tc.tile_pool(name="w8", bufs=3) as w8_pool,
                tc.tile_pool(name="qps", bufs=6, space="PSUM") as qps_pool,
                tc.tile_pool(name="rp", bufs=4) as rp_pool,
            ):
                xta = xta_pool.tile([P, KOP, NQ, 2, 512], f8, name="xta")
                cos_sb = xta_pool.tile([P, S], f16, name="cos_sb")
                sin_sb = xta_pool.tile([P, S], f16, name="sin_sb")
                bq_sb = xta_pool.tile([P, HL], f32, name="bq_sb")
                bk_sb = xta_pool.tile([P, HL], f32, name="bk_sb")
                first_w = w8_pool.tile([P, KOP, 2, D], f8, name="wsb",
                                       tag="w8")
                nc.sync.dma_start(
                    first_w[:], wq8[0].rearrange("p (kp i d) -> p kp i d",
                                                 i=2, d=D)
                )
                nc.sync.dma_start(bq_sb[:], bqs[:, :])
                for qc in range(NQ):
                    for kbp in range(KOP):
                        nc.sync.dma_start(xta[:, kbp, qc],
                                          xtA_r[:, kbp, qc])
                nc.sync.dma_start(cos_sb[:], cosP[:, :])
                nc.sync.dma_start(sin_sb[:], sinP[:, :])
                nc.sync.dma_start(bk_sb[:], bks[:, :])
                nc.sync.dma_start(ones_sb[:], ones_d[:, :])

                for h in range(HL):
                    for w_in, b_sb, dst in (
                        (wq8, bq_sb, qt_t), (wk8, bk_sb, kt_t)
                    ):
                        if h == 0 and w_in is wq8:
                            wsb = first_w
                        else:
                            wsb = w8_pool.tile([P, KOP, 2, D], f8, name="wsb",
                                               tag="w8")
                            nc.sync.dma_start(
                                wsb[:],
                                w_in[h].rearrange("p (kp i d) -> p kp i d",
                                                  i=2, d=D)
                            )
                        for qc in range(NQ):
                            sl = slice(qc * 512, (qc + 1) * 512)
                            ps = qps_pool.tile([P, 512], f32, name="qkps",
                                               tag="qkps")
                            for kbp in range(KOP):
                                nc.tensor.matmul(
                                    ps[:],
                                    lhsT=wsb[:, kbp],
                                    rhs=xta[:, kbp, qc],
                                    start=(kbp == 0),
                                    stop=(kbp == KOP - 1),
                                    perf_mode=DR,
                                )
                            # rope drain: qt = qb*cos + swap64(qb)*sinP
                            qb = rp_pool.tile([P, 512], f16, name="qb", tag="qb")
                            nc.scalar.activation(
                                qb[:], ps[:], AF.Identity, bias=b_sb[:, h:h + 1]
                            )
                            qsw = rp_pool.tile([P, 512], f16, name="qsw",
                                               tag="qsw")
                            nc.vector.tensor_copy(qsw[0:64], qb[64:128])
                            nc.vector.tensor_copy(qsw[64:128], qb[0:64])
                            t1 = rp_pool.tile([P, 512], f16, name="t1", tag="t1")
                            nc.vector.tensor_tensor(
                                t1[:], qb[:], cos_sb[:, sl], MULT
                            )
                            t2 = rp_pool.tile([P, 512], f16, name="t2", tag="t2")
                            nc.vector.tensor_tensor(
                                t2[:], qsw[:], sin_sb[:, sl], MULT
                            )
                            nc.vector.tensor_tensor(
                                dst[h][:, sl], t1[:], t2[:], ADD
                            )

            # ---------------- phase A: v projection (fp16) --------------
            # xtB's 8MB lands in the space phase B's pools vacate; loads are
            # interleaved (wv chunk, then that kb's first column group) so
            # the sb-major compute starts after ~2MB of traffic.
            with (
                tc.tile_pool(name="xtb", bufs=1) as xtb_pool,
                tc.tile_pool(name="wvp", bufs=2) as wv_pool,
                tc.tile_pool(name="vps", bufs=4, space="PSUM") as vps_pool,
            ):
                xtb = xtb_pool.tile([P, KO, S], f16, name="xtb")
                if with_bv:
                    bv_sb = wv_pool.tile([P, HL * D], f16, name="bv_sb",
                                         tag="bv")
                    nc.sync.dma_start(bv_sb[:], bv[:, :])
                wvt0 = wv_pool.tile([P, KO, 512], f16, name="wvt", tag="wv")
                for kb in range(KO):
                    nc.sync.dma_start(wvt0[:, kb], wv_r[kb][:, 0:512])
                    nc.sync.dma_start(xtb[:, kb, 0:512], xtB_r[:, kb, 0:512])
                for qc in range(1, NQ):
                    sl = slice(qc * 512, (qc + 1) * 512)
                    for kb in range(KO):
                        nc.sync.dma_start(xtb[:, kb, sl], xtB_r[:, kb, sl])
                for g2 in range(2):
                    if g2 == 0:
                        wvt = wvt0
                    else:
                        wvt = wv_pool.tile([P, KO, 512], f16, name="wvt",
                                           tag="wv")
                        for kb in range(KO):
                            nc.sync.dma_start(
                                wvt[:, kb], wv_r[kb][:, g2 * 512:(g2 + 1) * 512]
                            )
                    for sb in range(SB):
                        ps = vps_pool.tile([P, 512], f32, name="vps",
                                           tag="vps")
                        for kb in range(KO):
                            nc.tensor.matmul(
                                ps[:],
                                lhsT=xtb[:, kb, sb * P:(sb + 1) * P],
                                rhs=wvt[:, kb],
                                start=(kb == 0),
                                stop=(kb == KO - 1),
                            )
                        dstv = v_sb[:, sb, g2 * 512:(g2 + 1) * 512]
                        if with_bv:
                            nc.vector.tensor_tensor(
                                dstv, ps[:],
                                bv_sb[:, g2 * 512:(g2 + 1) * 512], ADD
                            )
                        else:
                            nc.vector.tensor_copy(dstv, ps[:])

            # ---------------- phase C: attention + o-proj ----------------
            with (
                tc.tile_pool(name="wores", bufs=1) as wo_pool,
                tc.tile_pool(name="mt", bufs=2) as m_pool,
                tc.tile_pool(name="prp", bufs=3) as pr_pool,
                tc.tile_pool(name="pmp", bufs=3) as pm_pool,
                tc.tile_pool(name="dap", bufs=2) as da_pool,
                tc.tile_pool(name="otq", bufs=2) as ot_pool,
                tc.tile_pool(name="odp", bufs=3) as od_pool,
                tc.tile_pool(name="rcp", bufs=2) as rc_pool,
                tc.tile_pool(name="scp", bufs=2, space="PSUM") as sc_pool,
                tc.tile_pool(name="avp", bufs=2, space="PSUM") as av_pool,
                tc.tile_pool(name="dnp", bufs=2, space="PSUM") as dn_pool,
            ):
                wo_sb = wo_pool.tile([P, HL, H], f16, name="wo_sb")

                def load_mask(qc):
                    mt = m_pool.tile([P, SB, 512], f16, name="mt", tag="mt")
                    sl = slice(qc * 512, (qc + 1) * 512)
                    for kb in range(KO):
                        nc.sync.dma_start(mt[:, kb], maskT_r[kb][:, sl])
                    return mt

                mt = load_mask(0)
                for qc in range(NQ):
                    sl = slice(qc * 512, (qc + 1) * 512)
                    oT_qc = ot_pool.tile([P, HL, 512], f16, name="oT", tag="oT")
                    pending = None

                    def flush_pending():
                        dacc_p, ps_av_p, hp, oT_p = pending
                        ps_dn = dn_pool.tile([P, 512], f32, name="dn", tag="dn")
                        for i in range(2):
                            nc.tensor.matmul(
                                ps_dn[:],
                                lhsT=ones_sb[:],
                                rhs=dacc_p[:, i],
                                start=(i == 0),
                                stop=(i == 1),
                            )
                        rc = rc_pool.tile([P, 512], f32, name="rc", tag="rc")
                        nc.vector.reciprocal_approx_fast(rc[:], ps_dn[:])
                        nc.vector.tensor_tensor(
                            oT_p[:, hp], ps_av_p[:], rc[:], MULT
                        )

                    for h in range(HL):
                        ps_av = av_pool.tile([P, 512], f32, name="av", tag="av")
                        dacc = da_pool.tile([P, 2, 512], f16, name="dacc",
                                            tag="dacc")
                        for kbp in range(KOP):
                            ps_s = sc_pool.tile([P, 2, 512], f32, name="ps_s",
                                                tag="ps_s")
                            for i in range(2):
                                kb = 2 * kbp + i
                                nc.tensor.matmul(
                                    ps_s[:, i],
                                    lhsT=kt_t[h][:, kb * P:(kb + 1) * P],
                                    rhs=qt_t[h][:, sl],
                                    start=True,
                                    stop=True,
                                )
                            pr = pr_pool.tile([P, 2, 512], f16, name="pr",
                                              tag="pr")
                            nc.scalar.activation(
                                pr[:], ps_s[:], AF.Exp, scale=EXP_SCALE
                            )
                            if kbp == 0:
                                nc.vector.tensor_copy(dacc[:], pr[:])
                            else:
                                nc.vector.tensor_tensor(
                                    dacc[:], dacc[:], pr[:], ADD
                                )
                            pm = pm_pool.tile([P, 2, 512], f16, name="pm",
                                              tag="pm")
                            nc.vector.tensor_tensor(
                                pm[:], pr[:], mt[:, 2 * kbp:2 * kbp + 2, :], MULT
                            )
                            for i in range(2):
                                kb = 2 * kbp + i
                                nc.tensor.matmul(
                                    ps_av[:],
                                    lhsT=v_sb[:, kb, h * D:(h + 1) * D],
                                    rhs=pm[:, i],
                                    start=(kbp == 0 and i == 0),
                                    stop=(kbp == KOP - 1 and i == 1),
                                )
                            # delayed denominator for the previous head so the
                            # PE never waits on the DVE accumulation chain
                            if kbp == 2 and pending is not None:
                                flush_pending()
                                pending = None
                            if kbp == 1 and h == 1 and qc == 0:
                                for hw in range(HL):
                                    nc.sync.dma_start(wo_sb[:, hw],
                                                      wo_r[:, hw, :])
                            if kbp == 3 and h == 1 and qc + 1 < NQ:
                                mt_next = load_mask(qc + 1)
                        pending = (dacc, ps_av, h, oT_qc)
                    flush_pending()
                    pending = None

                    # o-proj for row-blocks 4qc..4qc+3
                    for mm in range(4):
                        m = 4 * qc + mm
                        msl = slice(mm * 128, (mm + 1) * 128)
                        for n2 in range(2):
                            ops = sc_pool.tile([P, 2, 512], f32, name="ops",
                                               tag="ps_s")
                            for h in range(HL):
                                for i in range(2):
                                    ncol = (n2 * 2 + i) * 512
                                    nc.tensor.matmul(
                                        ops[:, i],
                                        lhsT=oT_qc[:, h, msl],
                                        rhs=wo_sb[:, h, ncol:ncol + 512],
                                        start=(h == 0),
                                        stop=(h == HL - 1),
                                    )
                            od = od_pool.tile([P, 2, 512], f16, name="od",
                                              tag="od")
                            nc.vector.tensor_copy(od[:], ops[:])
                            nc.gpsimd.dma_start(
                                out_r[m][:, n2 * 1024:(n2 + 1) * 1024], od[:]
                            )
                    if qc + 1 < NQ:
                        mt = mt_next

    nc.compile()
    return nc


def _get_program(with_bv):
    key = ("nc", with_bv)
    if key not in _CACHE:
        _CACHE[key] = _build_program(with_bv)
    return _CACHE[key]


def _host_inputs(x, attention_mask, Wq, bq, Wk, bk, Wv, bv, Wo, bo, with_bv):
    """Build the 8 per-core input maps (core = batch*2 + head_group)."""
    import ml_dtypes

    f8 = ml_dtypes.float8_e4m3
    perm = np.concatenate([np.arange(0, D, 2), np.arange(1, D, 2)])

    inv = (1.0 / (ROPE_BASE ** (np.arange(0, D, 2, dtype=np.float64) / D)))
    t = np.arange(S, dtype=np.float64)
    fr = inv[:, None] * t[None, :]          # (64, S)
    cosP = np.concatenate([np.cos(fr), np.cos(fr)], 0).astype(np.float16)
    # sign folded in: rope = q*cos + swap(q)*sinP with sinP negative on the
    # first 64 partitions (rope[0:64] = q[0:64]c - q[64:128]s)
    sinP = np.concatenate([-np.sin(fr), np.sin(fr)], 0).astype(np.float16)
    ones = np.ones((P, P), np.float16)

    def q8(a):
        return np.clip(a, -240.0, 240.0).astype(f8)

    def w_heads_fp8(W, g):
        # (HL, P, KOP*2*D) fp8, x32, rope-permuted, DoubleRow pair layout
        Wg = (W[:, g * HL * D:(g + 1) * HL * D] * WSCALE).reshape(H, HL, D)
        Wg = Wg[:, :, perm]                                # (H, HL, D)
        Wg = Wg.reshape(KOP, 2, P, HL, D).transpose(3, 2, 0, 1, 4)
        return np.ascontiguousarray(
            q8(Wg.reshape(HL, P, KOP * 2 * D))
        )

    def b_heads_perm(b, g):
        bg = (b[g * HL * D:(g + 1) * HL * D] * WSCALE).reshape(HL, D)
        return np.ascontiguousarray(bg[:, perm].T).astype(np.float32)

    groups = []
    for g in range(G):
        groups.append({
            "wq8": w_heads_fp8(Wq, g),
            "wk8": w_heads_fp8(Wk, g),
            "bqs": b_heads_perm(bq, g),
            "bks": b_heads_perm(bk, g),
            "wv": np.ascontiguousarray(
                Wv[:, g * HL * D:(g + 1) * HL * D].astype(np.float16)
            ),
            "bv": np.ascontiguousarray(np.broadcast_to(
                bv[g * HL * D:(g + 1) * HL * D], (P, HL * D)
            )).astype(np.float16),
            "wo": np.ascontiguousarray(
                Wo[g * HL * D:(g + 1) * HL * D, :]
                .reshape(HL, D, H).transpose(1, 0, 2).reshape(P, HL * H)
                .astype(np.float16)
            ),
        })

    in_maps = []
    for b in range(B):
        xT = x[b].T                                        # (H, S)
        xtA = np.ascontiguousarray(
            q8(xT.reshape(KOP, 2, P, NQ, 512).transpose(2, 0, 3, 1, 4)
               .reshape(P, KOP * 2 * S))
        )
        xtB = np.ascontiguousarray(
            xT.reshape(KO, P, S).transpose(1, 0, 2)
            .reshape(P, KO * S).astype(np.float16)
        )
        maskT = np.ascontiguousarray(
            attention_mask[b, 0].T.astype(np.float16)
        )
        for g in range(G):
            m = dict(groups[g])
            m["xtA"] = xtA
            m["xtB"] = xtB
            m["maskT"] = maskT
            m["cosP"] = cosP
            m["sinP"] = sinP
            m["ones"] = ones
            in_maps.append(m)
    return in_maps


def kernel(x, attention_mask, Wq, bq, Wk, bk, Wv, bv, Wo, bo, _trace=False,
           _tmpdir=None):
    from concourse.bass_utils import run_bass_kernel_spmd

    with_bv = bool(np.any(bv))
    nc = _get_program(with_bv)
    in_maps = _host_inputs(
        x, attention_mask, Wq, bq, Wk, bk, Wv, bv, Wo, bo, with_bv
    )
    res = run_bass_kernel_spmd(
        nc, in_maps, list(range(8)), trace=_trace, tmpdir=_tmpdir
    )
    outs = [res.results[c]["out"] for c in range(8)]
    full = np.empty((B, S, H), np.float32)
    for b in range(B):
        full[b] = (outs[2 * b].astype(np.float32)
                   + outs[2 * b + 1].astype(np.float32) + bo[None, :])
    if _trace:
        _CACHE["last_exec_time_ns"] = res.exec_time_ns
        _CACHE["last_results"] = res
    return full


# revision 10
# speedup vs baseline: 1.2090x; 1.2090x over previous
"""Trainium2 Bass kernel for the MultiLatentAttention (dense transformer) block.

Computes, for x:(4,2048,2048), mask:(4,1,2048,2048):
    q/k/v = x @ W{q,k,v} + b  (per-head, head_dim=128, 16 heads)
    q,k <- interleaved RoPE
    attn = softmax(q k^T / sqrt(2048)) * mask
    out  = (attn @ v) @ Wo + bo

Sharding: 8 cores = 4 batches x 2 head-groups (8 heads each). Each core
computes its batch's q/k/v for its 8 heads, attention, and a partial
o-projection (row-parallel over Wo). Host sums the two partials per batch
and adds bo. No device collectives.

Numerics / layout:
 - q/k projections run in fp8e4 with perf_mode=DoubleRow (256-deep
   contraction per pass, ~1.8x the fp16 matmul rate). Wq/Wk are scaled
   x32 on host so their values sit in e4m3's normal range; the x1024
   scores scale is folded into the exp() scale. Simulated end-to-end
   max-rel-err of this scheme is 1.3e-2 (gate: 2e-2); everything else
   runs in fp16 which alone sims at 5.4e-4.
 - v projection / scores / attn@v / o-projection all use fp16 operands
   (fp32 PSUM accumulate). fp16 halves LDWEIGHTS time vs fp32r and all
   SBUF/DMA traffic.
 - RoPE interleaved pairs are de-interleaved by permuting W{q,k} columns
   per head (even dims -> partitions 0..63, odd -> 64..127); q.k is
   invariant under a shared head-dim permutation and v/Wo are left
   unpermuted. Rotation sign is folded into the sin table.
 - softmax has no max-subtraction (scores are O(1)); the denominator is
   accumulated across k-blocks on DVE in fp16, then one pair of
   accumulating ones-stationary matmuls per (qc, head) produces the
   partition-broadcast sum (vs baseline's 16 ones-matmuls).
 - Everything stays resident in SBUF: q/k (fp16, 8MB), v (fp16, 4MB),
   per-qc mask tiles and attention outputs. Only the fp16 output
   partials are spilled (SWDGE so stores never block the load queue).
 - Attention loops qc (512 query cols) outer, heads inner; the o-proj
   for those 4 row-blocks runs right after each qc, giving the scalar
   engine (exp) slack to catch up while the PE does matmuls with no
   activation dependency.
"""

import numpy as np

B, S, H, NH = 4, 2048, 2048, 16
D = 128            # head dim
G = 2              # head groups (tensor-parallel)
HL = NH // G       # heads per core = 8
P = 128
KO = H // P        # 16 contraction blocks
KOP = KO // 2      # 8 DoubleRow contraction pairs
SB = S // P        # 16 sequence blocks
NQ = S // 512      # 4 query-column chunks
ROPE_BASE = 10000.0
WSCALE = 32.0      # host scale on Wq/Wk (and bq/bk) for fp8 range
SCALE = 1.0 / np.sqrt(np.float32(H))
EXP_SCALE = float(SCALE / (WSCALE * WSCALE))

_CACHE = {}


def _build_program(with_bv):
    import concourse.mybir as mybir
    import concourse.tile as tile
    from concourse import bacc

    f32 = mybir.dt.float32
    f16 = mybir.dt.float16
    f8 = mybir.dt.float8e4
    AF = mybir.ActivationFunctionType
    DR = mybir.MatmulPerfMode.DoubleRow
    ADD = mybir.AluOpType.add
    MULT = mybir.AluOpType.mult

    nc = bacc.Bacc("TRN2", num_devices=8, debug=False, num_swdge_queues=4)

    xtA = nc.dram_tensor("xtA", [P, KOP * 2 * S], f8, kind="ExternalInput")
    xtB = nc.dram_tensor("xtB", [P, KO * S], f16, kind="ExternalInput")
    wq8 = nc.dram_tensor("wq8", [HL, P, KOP * 2 * D], f8, kind="ExternalInput")
    wk8 = nc.dram_tensor("wk8", [HL, P, KOP * 2 * D], f8, kind="ExternalInput")
    bqs = nc.dram_tensor("bqs", [P, HL], f32, kind="ExternalInput")
    bks = nc.dram_tensor("bks", [P, HL], f32, kind="ExternalInput")
    wv = nc.dram_tensor("wv", [H, HL * D], f16, kind="ExternalInput")
    bv = nc.dram_tensor("bv", [P, HL * D], f16, kind="ExternalInput")
    wo = nc.dram_tensor("wo", [P, HL * H], f16, kind="ExternalInput")
    cosP = nc.dram_tensor("cosP", [P, S], f16, kind="ExternalInput")
    sinP = nc.dram_tensor("sinP", [P, S], f16, kind="ExternalInput")
    maskT = nc.dram_tensor("maskT", [S, S], f16, kind="ExternalInput")
    ones_d = nc.dram_tensor("ones", [P, P], f16, kind="ExternalInput")

    out = nc.dram_tensor("out", [S, H], f16, kind="ExternalOutput")

    xtA_r = xtA.rearrange("p (kp q i c) -> p kp q i c", kp=KOP, q=NQ, i=2)
    xtB_r = xtB.rearrange("p (ko s) -> p ko s", s=S)
    wv_r = wv.rearrange("(ko p) n -> ko p n", p=P)
    wo_r = wo.rearrange("p (h n) -> p h n", n=H)
    maskT_r = maskT.rearrange("(ko p) s -> ko p s", p=P)
    out_r = out.rearrange("(mo p) n -> mo p n", p=P)

    with tile.TileContext(nc) as tc:
        with (
            tc.tile_pool(name="vres", bufs=1) as vres_pool,
            tc.tile_pool(name="qkres", bufs=1) as qkres_pool,
            tc.tile_pool(name="cs", bufs=1) as cs_pool,
        ):
            v_sb = vres_pool.tile([P, SB, HL * D], f16, name="v_sb")
            qt_t = [
                qkres_pool.tile([P, S], f16, name=f"qt{h}", tag=f"qt{h}")
                for h in range(HL)
            ]
            kt_t = [
                qkres_pool.tile([P, S], f16, name=f"kt{h}", tag=f"kt{h}")
                for h in range(HL)
            ]
            ones_sb = cs_pool.tile([P, P], f16, name="ones_sb")

            # ---------------- phase B: q/k projections (fp8 DoubleRow) ----
            with (
                tc.tile_pool(name="xta", bufs=1) as xta_pool,
                tc.tile_pool(name="w8", bufs=3) as w8_pool,
                tc.tile_pool(name="qps", bufs=6, space="PSUM") as qps_pool,
                tc.tile_pool(name="rp", bufs=4) as rp_pool,
            ):
                xta = xta_pool.tile([P, KOP, NQ, 2, 512], f8, name="xta")
                cos_sb = xta_pool.tile([P, S], f16, name="cos_sb")
                sin_sb = xta_pool.tile([P, S], f16, name="sin_sb")
                bq_sb = xta_pool.tile([P, HL], f32, name="bq_sb")
                bk_sb = xta_pool.tile([P, HL], f32, name="bk_sb")
                first_w = w8_pool.tile([P, KOP, 2, D], f8, name="wsb",
                                       tag="w8")
                nc.sync.dma_start(
                    first_w[:], wq8[0].rearrange("p (kp i d) -> p kp i d",
                                                 i=2, d=D)
                )
                nc.sync.dma_start(bq_sb[:], bqs[:, :])
                for qc in range(NQ):
                    for kbp in range(KOP):
                        nc.sync.dma_start(xta[:, kbp, qc],
                                          xtA_r[:, kbp, qc])
                nc.sync.dma_start(cos_sb[:], cosP[:, :])
                nc.sync.dma_start(sin_sb[:], sinP[:, :])
                nc.sync.dma_start(bk_sb[:], bks[:, :])
                nc.sync.dma_start(ones_sb[:], ones_d[:, :])

                for h in range(HL):
                    for w_in, b_sb, dst in (
                        (wq8, bq_sb, qt_t), (wk8, bk_sb, kt_t)
                    ):
                        if h == 0 and w_in is wq8:
                            wsb = first_w
                        else:
                            wsb = w8_pool.tile([P, KOP, 2, D], f8, name="wsb",
                                               tag="w8")
                            nc.sync.dma_start(
                                wsb[:],
                                w_in[h].rearrange("p (kp i d) -> p kp i d",
                                                  i=2, d=D)
                            )
                        for qc in range(NQ):
                            sl = slice(qc * 512, (qc + 1) * 512)
                            ps = qps_pool.tile([P, 512], f32, name="qkps",
                                               tag="qkps")
                            for kbp in range(KOP):
                                nc.tensor.matmul(
                                    ps[:],
                                    lhsT=wsb[:, kbp],
                                    rhs=xta[:, kbp, qc],
                                    start=(kbp == 0),
                                    stop=(kbp == KOP - 1),
                                    perf_mode=DR,
                                )
                            # rope drain: qt = qb*cos + swap64(qb)*sinP
                            qb = rp_pool.tile([P, 512], f16, name="qb", tag="qb")
                            nc.scalar.activation(
                                qb[:], ps[:], AF.Identity, bias=b_sb[:, h:h + 1]
                            )
                            qsw = rp_pool.tile([P, 512], f16, name="qsw",
                                               tag="qsw")
                            nc.scalar.copy(qsw[0:64], qb[64:128])
                            nc.vector.tensor_copy(qsw[64:128], qb[0:64])
                            t1 = rp_pool.tile([P, 512], f16, name="t1", tag="t1")
                            nc.vector.tensor_tensor(
                                t1[:], qb[:], cos_sb[:, sl], MULT
                            )
                            t2 = rp_pool.tile([P, 512], f16, name="t2", tag="t2")
                            nc.vector.tensor_tensor(
                                t2[:], qsw[:], sin_sb[:, sl], MULT
                            )
                            nc.vector.tensor_tensor(
                                dst[h][:, sl], t1[:], t2[:], ADD
                            )

            # ---------------- phase A: v projection (fp16) --------------
            # xtB's 8MB lands in the space phase B's pools vacate; loads are
            # interleaved (wv chunk, then that kb's first column group) so
            # the sb-major compute starts after ~2MB of traffic.
            with (
                tc.tile_pool(name="xtb", bufs=1) as xtb_pool,
                tc.tile_pool(name="wvp", bufs=2) as wv_pool,
                tc.tile_pool(name="vps", bufs=4, space="PSUM") as vps_pool,
            ):
                xtb = xtb_pool.tile([P, KO, S], f16, name="xtb")
                if with_bv:
                    bv_sb = wv_pool.tile([P, HL * D], f16, name="bv_sb",
                                         tag="bv")
                    nc.sync.dma_start(bv_sb[:], bv[:, :])
                wvt0 = wv_pool.tile([P, KO, 512], f16, name="wvt", tag="wv")
                for kb in range(KO):
                    nc.sync.dma_start(wvt0[:, kb], wv_r[kb][:, 0:512])
                    nc.sync.dma_start(xtb[:, kb, 0:512], xtB_r[:, kb, 0:512])
                for qc in range(1, NQ):
                    sl = slice(qc * 512, (qc + 1) * 512)
                    for kb in range(KO):
                        nc.sync.dma_start(xtb[:, kb, sl], xtB_r[:, kb, sl])
                for g2 in range(2):
                    if g2 == 0:
                        wvt = wvt0
                    else:
                        wvt = wv_pool.tile([P, KO, 512], f16, name="wvt",
                                           tag="wv")
                        for kb in range(KO):
                            nc.sync.dma_start(
                                wvt[:, kb], wv_r[kb][:, g2 * 512:(g2 + 1) * 512]
                            )
                    for sb in range(SB):
                        ps = vps_pool.tile([P, 512], f32, name="vps",
                                           tag="vps")
                        for kb in range(KO):
                            nc.tensor.matmul(
                                ps[:],
                                lhsT=xtb[:, kb, sb * P:(sb + 1) * P],
                                rhs=wvt[:, kb],
                                start=(kb == 0),
                                stop=(kb == KO - 1),
                            )
                        dstv = v_sb[:, sb, g2 * 512:(g2 + 1) * 512]
                        if with_bv:
                            nc.vector.tensor_tensor(
                                dstv, ps[:],
                                bv_sb[:, g2 * 512:(g2 + 1) * 512], ADD
                            )
                        else:
                            nc.vector.tensor_copy(dstv, ps[:])

            # ---------------- phase C: attention + o-proj ----------------
            with (
                tc.tile_pool(name="wores", bufs=1) as wo_pool,
                tc.tile_pool(name="mt", bufs=2) as m_pool,
                tc.tile_pool(name="prp", bufs=3) as pr_pool,
                tc.tile_pool(name="pmp", bufs=3) as pm_pool,
                tc.tile_pool(name="dap", bufs=2) as da_pool,
                tc.tile_pool(name="otq", bufs=2) as ot_pool,
                tc.tile_pool(name="odp", bufs=3) as od_pool,
                tc.tile_pool(name="rcp", bufs=2) as rc_pool,
                tc.tile_pool(name="scp", bufs=2, space="PSUM") as sc_pool,
                tc.tile_pool(name="avp", bufs=2, space="PSUM") as av_pool,
                tc.tile_pool(name="dnp", bufs=2, space="PSUM") as dn_pool,
            ):
                wo_sb = wo_pool.tile([P, HL, H], f16, name="wo_sb")

                def load_mask(qc):
                    mt = m_pool.tile([P, SB, 512], f16, name="mt", tag="mt")
                    sl = slice(qc * 512, (qc + 1) * 512)
                    for kb in range(KO):
                        nc.sync.dma_start(mt[:, kb], maskT_r[kb][:, sl])
                    return mt

                mt = load_mask(0)
                for qc in range(NQ):
                    sl = slice(qc * 512, (qc + 1) * 512)
                    oT_qc = ot_pool.tile([P, HL, 512], f16, name="oT", tag="oT")
                    pending = None

                    def flush_pending():
                        dacc_p, ps_av_p, hp, oT_p = pending
                        ps_dn = dn_pool.tile([P, 512], f32, name="dn", tag="dn")
                        for i in range(2):
                            nc.tensor.matmul(
                                ps_dn[:],
                                lhsT=ones_sb[:],
                                rhs=dacc_p[:, i],
                                start=(i == 0),
                                stop=(i == 1),
                            )
                        rc = rc_pool.tile([P, 512], f32, name="rc", tag="rc")
                        nc.vector.reciprocal_approx_fast(rc[:], ps_dn[:])
                        nc.vector.tensor_tensor(
                            oT_p[:, hp], ps_av_p[:], rc[:], MULT
                        )

                    for h in range(HL):
                        ps_av = av_pool.tile([P, 512], f32, name="av", tag="av")
                        dacc = da_pool.tile([P, 2, 512], f16, name="dacc",
                                            tag="dacc")
                        for kbp in range(KOP):
                            ps_s = sc_pool.tile([P, 2, 512], f32, name="ps_s",
                                                tag="ps_s")
                            for i in range(2):
                                kb = 2 * kbp + i
                                nc.tensor.matmul(
                                    ps_s[:, i],
                                    lhsT=kt_t[h][:, kb * P:(kb + 1) * P],
                                    rhs=qt_t[h][:, sl],
                                    start=True,
                                    stop=True,
                                )
                            pr = pr_pool.tile([P, 2, 512], f16, name="pr",
                                              tag="pr")
                            nc.scalar.activation(
                                pr[:], ps_s[:], AF.Exp, scale=EXP_SCALE
                            )
                            if kbp == 0:
                                nc.vector.tensor_copy(dacc[:], pr[:])
                            else:
                                nc.vector.tensor_tensor(
                                    dacc[:], dacc[:], pr[:], ADD
                                )
                            pm = pm_pool.tile([P, 2, 512], f16, name="pm",
                                              tag="pm")
                            nc.vector.tensor_tensor(
                                pm[:], pr[:], mt[:, 2 * kbp:2 * kbp + 2, :], MULT
                            )
                            for i in range(2):
                                kb = 2 * kbp + i
                                nc.tensor.matmul(
                                    ps_av[:],
                                    lhsT=v_sb[:, kb, h * D:(h + 1) * D],
                                    rhs=pm[:, i],
                                    start=(kbp == 0 and i == 0),
                                    stop=(kbp == KOP - 1 and i == 1),
                                )
                            # delayed denominator for the previous head so the
                            # PE never waits on the DVE accumulation chain
                            if kbp == 2 and pending is not None:
                                flush_pending()
                                pending = None
                            if kbp == 1 and h == 1 and qc == 0:
                                for hw in range(HL):
                                    nc.sync.dma_start(wo_sb[:, hw],
                                                      wo_r[:, hw, :])
                            if kbp == 3 and h == 1 and qc + 1 < NQ:
                                mt_next = load_mask(qc + 1)
                        pending = (dacc, ps_av, h, oT_qc)
                    flush_pending()
                    pending = None

                    # o-proj for row-blocks 4qc..4qc+3
                    for mm in range(4):
                        m = 4 * qc + mm
                        msl = slice(mm * 128, (mm + 1) * 128)
                        for n2 in range(2):
                            ops = sc_pool.tile([P, 2, 512], f32, name="ops",
                                               tag="ps_s")
                            for h in range(HL):
                                for i in range(2):
                                    ncol = (n2 * 2 + i) * 512
                                    nc.tensor.matmul(
                                        ops[:, i],
                                        lhsT=oT_qc[:, h, msl],
                                        rhs=wo_sb[:, h, ncol:ncol + 512],
                                        start=(h == 0),
                                        stop=(h == HL - 1),
                                    )
                            od = od_pool.tile([P, 2, 512], f16, name="od",
                                              tag="od")
                            nc.scalar.activation(od[:], ops[:], AF.Copy)
                            nc.gpsimd.dma_start(
                                out_r[m][:, n2 * 1024:(n2 + 1) * 1024], od[:]
                            )
                    if qc + 1 < NQ:
                        mt = mt_next

    nc.compile()
    return nc


def _get_program(with_bv):
    key = ("nc", with_bv)
    if key not in _CACHE:
        _CACHE[key] = _build_program(with_bv)
    return _CACHE[key]


def _host_inputs(x, attention_mask, Wq, bq, Wk, bk, Wv, bv, Wo, bo, with_bv):
    """Build the 8 per-core input maps (core = batch*2 + head_group)."""
    import ml_dtypes

    f8 = ml_dtypes.float8_e4m3
    perm = np.concatenate([np.arange(0, D, 2), np.arange(1, D, 2)])

    inv = (1.0 / (ROPE_BASE ** (np.arange(0, D, 2, dtype=np.float64) / D)))
    t = np.arange(S, dtype=np.float64)
    fr = inv[:, None] * t[None, :]          # (64, S)
    cosP = np.concatenate([np.cos(fr), np.cos(fr)], 0).astype(np.float16)
    # sign folded in: rope = q*cos + swap(q)*sinP with sinP negative on the
    # first 64 partitions (rope[0:64] = q[0:64]c - q[64:128]s)
    sinP = np.concatenate([-np.sin(fr), np.sin(fr)], 0).astype(np.float16)
    ones = np.ones((P, P), np.float16)

    def q8(a):
        return np.clip(a, -240.0, 240.0).astype(f8)

    def w_heads_fp8(W, g):
        # (HL, P, KOP*2*D) fp8, x32, rope-permuted, DoubleRow pair layout
        Wg = (W[:, g * HL * D:(g + 1) * HL * D] * WSCALE).reshape(H, HL, D)
        Wg = Wg[:, :, perm]                                # (H, HL, D)
        Wg = Wg.reshape(KOP, 2, P, HL, D).transpose(3, 2, 0, 1, 4)
        return np.ascontiguousarray(
            q8(Wg.reshape(HL, P, KOP * 2 * D))
        )

    def b_heads_perm(b, g):
        bg = (b[g * HL * D:(g + 1) * HL * D] * WSCALE).reshape(HL, D)
        return np.ascontiguousarray(bg[:, perm].T).astype(np.float32)

    groups = []
    for g in range(G):
        groups.append({
            "wq8": w_heads_fp8(Wq, g),
            "wk8": w_heads_fp8(Wk, g),
            "bqs": b_heads_perm(bq, g),
            "bks": b_heads_perm(bk, g),
            "wv": np.ascontiguousarray(
                Wv[:, g * HL * D:(g + 1) * HL * D].astype(np.float16)
            ),
            "bv": np.ascontiguousarray(np.broadcast_to(
                bv[g * HL * D:(g + 1) * HL * D], (P, HL * D)
            )).astype(np.float16),
            "wo": np.ascontiguousarray(
                Wo[g * HL * D:(g + 1) * HL * D, :]
                .reshape(HL, D, H).transpose(1, 0, 2).reshape(P, HL * H)
                .astype(np.float16)
            ),
        })

    in_maps = []
    for b in range(B):
        xT = x[b].T                                        # (H, S)
        xtA = np.ascontiguousarray(
            q8(xT.reshape(KOP, 2, P, NQ, 512).transpose(2, 0, 3, 1, 4)
               .reshape(P, KOP * 2 * S))
        )
        xtB = np.ascontiguousarray(
            xT.reshape(KO, P, S).transpose(1, 0, 2)
            .reshape(P, KO * S).astype(np.float16)
        )
        maskT = np.ascontiguousarray(
            attention_mask[b, 0].T.astype(np.float16)
        )
        for g in range(G):
            m = dict(groups[g])
            m["xtA"] = xtA
            m["xtB"] = xtB
            m["maskT"] = maskT
            m["cosP"] = cosP
            m["sinP"] = sinP
            m["ones"] = ones
            in_maps.append(m)
    return in_maps


def kernel(x, attention_mask, Wq, bq, Wk, bk, Wv, bv, Wo, bo, _trace=False,
           _tmpdir=None):
    from concourse.bass_utils import run_bass_kernel_spmd

    with_bv = bool(np.any(bv))
    nc = _get_program(with_bv)
    in_maps = _host_inputs(
        x, attention_mask, Wq, bq, Wk, bk, Wv, bv, Wo, bo, with_bv
    )
    res = run_bass_kernel_spmd(
        nc, in_maps, list(range(8)), trace=_trace, tmpdir=_tmpdir
    )
    outs = [res.results[c]["out"] for c in range(8)]
    full = np.empty((B, S, H), np.float32)
    for b in range(B):
        full[b] = (outs[2 * b].astype(np.float32)
                   + outs[2 * b + 1].astype(np.float32) + bo[None, :])
    if _trace:
        _CACHE["last_exec_time_ns"] = res.exec_time_ns
        _CACHE["last_results"] = res
    return full
